# revision 31
# baseline (speedup 1.0000x reference)
"""BasicAttention Trainium2 kernel.

Reference computation (per batch b):
    q = x[b] @ Wq + bq            # [S, D]
    k = x[b] @ Wk + bk            # [S, D]
    v = x[b] @ Wv + bv            # [S, D]
    s = q @ k.T / QD              # [S, S]
    w = softmax(where(mask==0, -inf, s))
    out[b] = w @ v                # [S, D]

Sharding: 8 cores = 4 batches x 2 query-halves. Each core computes K/V for
its full batch (2048 keys) plus attention for its 1024-query half. SPMD, no
collectives. The program always treats rows [0:Sq] of its x input as the
queries; for odd cores the host rotates the key axis (and mask columns) by
Sq so their query half lands at the front — softmax and P@V are invariant
to key order.

Per-core kernel (all matmuls bf16 with fp32 PSUM accumulate):
  - x row-tiles cast-DMA'd f32->bf16 by SWDGE straight into SBUF and
    PE-transposed (bf16, 8 sub-blocks batched per PSUM bank) into x^T;
    query-half tiles first so QT starts ~10us in
  - mask cast int32->bf16 via SWDGE to DRAM scratch, xbar-DMA-transposed
    per key-tile for the scores phase
  - Wq/Wk/Wv loaded as contiguous per-e-chunk panels (scalar HWDGE queue,
    0.5MB each) + DVE cast into ONE resident bf16 W tile reused across the
    three projections (strided d-tile loads measured ~24GB/s — avoid)
  - QT[d, q] / KT[d, s] projections: weights stationary, x^T moving
  - V[s, d] natural: x^T tiles stationary, Wv moving; bv via rank-1 (K=1)
    matmul accumulation
  - scores computed TRANSPOSED: ST[ks, q] = KT-stationary @ QT-moving, so
    the softmax mask multiply is a plain elementwise op in [ks, q] layout
    and P never needs an on-chip transpose
  - exp on ACT (scale=1/QD fused), mask multiply on DVE
  - denominator: ones-column matmul with P^T stationary -> denomT [q, 1]
    in per-partition layout; reciprocal on DVE
  - out = (P^T.T @ V) scaled by 1/denom on PSUM eviction (ACT), f32 out
No row-max subtraction: scores/QD are within [-0.1, 0.1] so exp is safe,
and softmax is shift-invariant, matching the reference exactly.
"""

import sys

if "/opt/trn_rl_repo" not in sys.path:
    sys.path.insert(0, "/opt/trn_rl_repo")

import numpy as np

B, S_FULL, E_DIM, QD = 4, 2048, 1024, 1024
N_CORES = 8
P = 128
INV_QD = 1.0 / 1024.0  # reference divides scores by QD=1024


def _chunks(total, step):
    out = []
    c = 0
    while c < total:
        out.append((c, min(step, total - c)))
        c += step
    return out


def build_nc(S=2048, Sq=1024, E=1024, D=1024):
    """Build + compile the per-core Bass program."""
    from contextlib import ExitStack

    import concourse.tile as tile
    from concourse import bacc, mybir

    bf16 = mybir.dt.bfloat16
    f32 = mybir.dt.float32
    i32 = mybir.dt.int32
    AF = mybir.ActivationFunctionType
    ALU = mybir.AluOpType

    NE = E // P    # e-chunks (contraction tiles for projections)
    ND = D // P    # d-tiles
    NS = S // P    # key tiles
    NQ = Sq // P   # query tiles
    NCH = 512      # matmul moving-dim chunk (one fp32 PSUM bank)
    SLAB = 1024    # psum tile free width (2 banks)
    S2 = S // 2    # x cast granularity (column-half blocks)
    assert Sq <= SLAB and D <= SLAB

    from concourse.masks import make_identity

    nc = bacc.Bacc("TRN2", target_bir_lowering=False, debug=False)

    x_d = nc.dram_tensor("x", [S, E], f32, kind="ExternalInput").ap()
    mask_d = nc.dram_tensor("mask", [Sq, S], i32, kind="ExternalInput").ap()
    wq_d = nc.dram_tensor("Wq", [E, D], f32, kind="ExternalInput").ap()
    bq_d = nc.dram_tensor("bq", [D], f32, kind="ExternalInput").ap()
    wk_d = nc.dram_tensor("Wk", [E, D], f32, kind="ExternalInput").ap()
    bk_d = nc.dram_tensor("bk", [D], f32, kind="ExternalInput").ap()
    wv_d = nc.dram_tensor("Wv", [E, D], f32, kind="ExternalInput").ap()
    bv_d = nc.dram_tensor("bv", [D], f32, kind="ExternalInput").ap()
    out_d = nc.dram_tensor("out", [Sq, D], f32, kind="ExternalOutput").ap()

    with ExitStack() as ctx:
        tc = ctx.enter_context(tile.TileContext(nc))
        dram = ctx.enter_context(tc.tile_pool(name="dram", bufs=1, space="DRAM"))

        # ---- SBUF pools (all persistent; total ~23.7 MB) ----
        const = ctx.enter_context(tc.tile_pool(name="const", bufs=1))
        xt_pool = ctx.enter_context(tc.tile_pool(name="xt", bufs=1))
        xs_pool = ctx.enter_context(tc.tile_pool(name="xs", bufs=3))
        qt_pool = ctx.enter_context(tc.tile_pool(name="qt", bufs=1))
        kt_pool = ctx.enter_context(tc.tile_pool(name="kt", bufs=1))
        v_pool = ctx.enter_context(tc.tile_pool(name="v", bufs=1))
        pst_pool = ctx.enter_context(tc.tile_pool(name="pst", bufs=1))
        w_pool = ctx.enter_context(tc.tile_pool(name="w", bufs=2))
        wbf_pool = ctx.enter_context(tc.tile_pool(name="wbf", bufs=1))
        evict = ctx.enter_context(tc.tile_pool(name="evict", bufs=2))
        maskt_pool = ctx.enter_context(tc.tile_pool(name="maskt", bufs=2))
        o_pool = ctx.enter_context(tc.tile_pool(name="o", bufs=1))
        den_pool = ctx.enter_context(tc.tile_pool(name="den", bufs=2))

        # PSUM: shared matmul pool (3 x 2 banks) + denominator pool (2 x 1 bank)
        mm_psum = ctx.enter_context(tc.tile_pool(name="mm_psum", bufs=3, space="PSUM"))
        den_psum = ctx.enter_context(tc.tile_pool(name="den_psum", bufs=2, space="PSUM"))

        # constants (scalar-queue DMAs; tiny)
        ones_row = const.tile([1, P], bf16)           # rank-1 bias lhsT
        nc.vector.memset(ones_row[0:1, :], 1.0)
        ones_col = const.tile([P, 1], bf16)           # denominator rhs
        nc.vector.memset(ones_col[:, 0:1], 1.0)
        bqk_t = const.tile([P, 2 * ND], f32, name="bqk")  # bq cols | bk cols
        nc.scalar.dma_start(out=bqk_t[:, 0:ND], in_=bq_d.rearrange("(o p) -> p o", p=P))
        nc.scalar.dma_start(
            out=bqk_t[:, ND : 2 * ND], in_=bk_d.rearrange("(o p) -> p o", p=P)
        )
        bv_t = const.tile([1, D], bf16)
        nc.gpsimd.dma_start(out=bv_t[0:1, :], in_=bv_d.rearrange("(a d) -> a d", a=1))
        ident = const.tile([P, P], bf16)
        make_identity(nc, ident)
        ident32 = const.tile([P, P], f32)
        make_identity(nc, ident32)

        # big persistent tensors (bf16)
        xT = xt_pool.tile([P, NE, S], bf16)      # xT[p, e, s] = x[s, e*P+p]
        QT = qt_pool.tile([P, ND, Sq], bf16)     # QT[p, dt, q] = Q[q, dt*P+p]
        KT = kt_pool.tile([P, ND, S], bf16)      # KT[p, dt, s] = K[s, dt*P+p]
        V = v_pool.tile([P, NS, D], bf16)        # V[p, st, d] = V[st*P+p, d]
        PsT = pst_pool.tile([P, NS, Sq], bf16)   # P^T[p, kt, q]
        HD2 = D if D <= NCH else D // 2  # W half width (no split when D fits one bank)
        WbfA = wbf_pool.tile([P, NE, HD2], bf16)
        WbfB = wbf_pool.tile([P, NE, max(D - HD2, P)], bf16)

        # ---- phase 0: x row-tiles PE-transposed into x^T. Query-half tiles
        #      arrive via SWDGE cast-DMA (bf16 straight to SBUF); key-half
        #      tiles via HWDGE f32 loads + f32 transposes + DVE cast-copies —
        #      two parallel DMA channels. Query half first so QT can start;
        #      the key half interleaves with the QT d-tiles below. ----
        def load_transpose_xtile(st):
            # Most tiles: SWDGE cast-DMA (f32->bf16) to SBUF + bf16 PE
            # transposes. Tile 0 and the last key tiles ride the otherwise
            # idle HWDGE/f32 path so the PE starts sooner and the key half
            # finishes ~15us earlier than the SWDGE stream alone.
            if st == 0 or st >= NS - 4:
                x32 = xs_pool.tile([P, E], f32, tag="xs32", bufs=1)
                nc.sync.dma_start(out=x32[:, :], in_=x_d[st * P : (st + 1) * P, :])
                for eg in range(0, NE, 4):
                    ecnt = min(4, NE - eg)
                    tr = den_psum.tile([P, 4, P], f32, tag="den")
                    for el in range(ecnt):
                        nc.tensor.transpose(
                            tr[:, el, :],
                            x32[:, (eg + el) * P : (eg + el + 1) * P],
                            ident32,
                        )
                    nc.vector.tensor_copy(
                        xT[:, eg : eg + ecnt, st * P : (st + 1) * P],
                        tr[:, 0:ecnt, :],
                    )
            else:
                x16 = xs_pool.tile([P, E], bf16, tag="xs")
                nc.gpsimd.dma_start(out=x16[:, :], in_=x_d[st * P : (st + 1) * P, :])
                tr = den_psum.tile([P, NE, P], bf16, tag="den")
                for e in range(NE):
                    nc.tensor.transpose(
                        tr[:, e, :], x16[:, e * P : (e + 1) * P], ident
                    )
                nc.vector.tensor_copy(xT[:, :, st * P : (st + 1) * P], tr[:, :, :])

        def load_w_half(w_src, d0, dst_tile):
            # contiguous [P, HD2] f32 rows -> DVE cast into one W half-tile
            hw = min(HD2, D - d0)
            for e in range(NE):
                w32 = w_pool.tile([P, HD2], f32, tag="w32")
                nc.scalar.dma_start(
                    out=w32[:, 0:hw], in_=w_src[e * P : (e + 1) * P, d0 : d0 + hw]
                )
                nc.vector.tensor_copy(dst_tile[:, e, 0:hw], w32[:, 0:hw])

        def load_w_panels(w_src):
            load_w_half(w_src, 0, WbfA)
            if D > HD2:
                load_w_half(w_src, HD2, WbfB)

        def w_slice(e, dcol, width):
            # stationary slice [P, width] at global d-column dcol
            if dcol < HD2:
                return WbfA[:, e, dcol : dcol + width]
            return WbfB[:, e, dcol - HD2 : dcol - HD2 + width]

        # prefetch Wq panels before anything else on the scalar queue
        with nc.named_scope("wq"):
            load_w_panels(wq_d)
        with nc.named_scope("xT"):
            for st in range(NQ):  # query half first
                load_transpose_xtile(st)

        # ---- phase 1: QT and KT projections (weights stationary, x^T moving) ----
        for wi, (w_src, span, dst, scope) in enumerate(
            ((wq_d, Sq, QT, "QT"), (wk_d, S, KT, "KT"))
        ):
            with nc.named_scope(scope):
                if wi == 1:
                    load_w_panels(w_src)  # Wq was prefetched up front
                # d-tile blocks, e-outer: each arriving W panel feeds
                # block_dts x chunks matmuls instead of stalling per-e.
                # Blocks never straddle the W half boundary so the next
                # projection's half-loads unblock as early as possible.
                BDT = min(2, max(HD2 // P, 1)) if span <= SLAB else 1
                half_nd = HD2 // P
                for db in range(0, ND, BDT):
                    dts = [
                        dt for dt in range(db, min(db + BDT, ND))
                        if dt // half_nd == db // half_nd
                    ] or [db]
                    if wi == 0:
                        for dt in dts:
                            if NQ + dt < NS:
                                load_transpose_xtile(NQ + dt)
                    pss = {}
                    for dt in dts:
                        pss[dt] = []
                        for s0 in range(0, span, SLAB):
                            sw = min(SLAB, span - s0)
                            ps = mm_psum.tile([P, SLAB], f32, tag="mm")
                            pss[dt].append((s0, sw, ps))
                    for e in range(NE):
                        for dt in dts:
                            for s0, sw, ps in pss[dt]:
                                for c0, cw in _chunks(sw, NCH):
                                    nc.tensor.matmul(
                                        ps[:, c0 : c0 + cw],
                                        w_slice(e, dt * P, P),
                                        xT[:, e, s0 + c0 : s0 + c0 + cw],
                                        start=(e == 0),
                                        stop=(e == NE - 1),
                                    )
                    for dt in dts:
                        bias_ap = bqk_t[:, wi * ND + dt : wi * ND + dt + 1]
                        for s0, sw, ps in pss[dt]:
                            nc.scalar.activation(
                                dst[:, dt, s0 : s0 + sw],
                                ps[:, 0:sw],
                                AF.Identity,
                                bias=bias_ap,
                            )
                if wi == 1:
                    # any key-half x tiles the QT loop didn't cover
                    for st in range(min(NQ + ND, NS), NS):
                        load_transpose_xtile(st)

        # mask cast int32->bf16 scratch (SWDGE, after the x tiles in queue
        # order); needed from the scores phase onward
        mask_bf = dram.tile([Sq, S], bf16)
        with nc.named_scope("mcast"):
            for r in range(0, Sq, 256):
                nc.gpsimd.dma_start(
                    out=mask_bf[r : r + 256, :], in_=mask_d[r : r + 256, :]
                )

        # ---- phase 1b: V natural (x^T stationary, Wv moving, rank-1 bias) ----
        with nc.named_scope("V"):
            load_w_panels(wv_d)
            for st in range(NS):
                ps = mm_psum.tile([P, SLAB], f32, tag="mm")
                for e in range(NE):
                    for c0, cw in _chunks(D, NCH):
                        nc.tensor.matmul(
                            ps[:, c0 : c0 + cw],
                            xT[:, e, st * P : (st + 1) * P],
                            w_slice(e, c0, cw),
                            start=(e == 0),
                            stop=False,
                        )
                for c0, cw in _chunks(D, NCH):
                    nc.tensor.matmul(
                        ps[:, c0 : c0 + cw],
                        ones_row[0:1, :],
                        bv_t[0:1, c0 : c0 + cw],
                        start=False,
                        stop=True,
                    )
                nc.scalar.copy(V[:, st, :], ps[:, 0:D])

        # ---- phase 2: transposed scores + softmax numerator ----
        with nc.named_scope("scores"):
            for kt in range(NS):
                mt = maskt_pool.tile([P, Sq], bf16, tag="maskt")
                nc.sync.dma_start(
                    out=mt[:, :],
                    in_=mask_bf[:, kt * P : (kt + 1) * P],
                    transpose=True,
                )
                ps = mm_psum.tile([P, SLAB], f32, tag="mm")
                for dt in range(ND):
                    for c0, cw in _chunks(Sq, NCH):
                        nc.tensor.matmul(
                            ps[:, c0 : c0 + cw],
                            KT[:, dt, kt * P : (kt + 1) * P],
                            QT[:, dt, c0 : c0 + cw],
                            start=(dt == 0),
                            stop=(dt == ND - 1),
                        )
                ex = evict.tile([P, Sq], bf16, tag="exp")
                nc.scalar.activation(ex[:, :], ps[:, 0:Sq], AF.Exp, scale=INV_QD)
                nc.vector.tensor_tensor(
                    PsT[:, kt, :], ex[:, :], mt[:, :], op=ALU.mult
                )

        # ---- phase 3: denominator + P@V per query tile ----
        with nc.named_scope("pv"):
            for qt in range(NQ):
                dps = den_psum.tile([P, 1], f32, tag="den")
                ops = mm_psum.tile([P, SLAB], f32, tag="mm")
                for kt in range(NS):
                    pst_tile = PsT[:, kt, qt * P : (qt + 1) * P]
                    nc.tensor.matmul(
                        dps[:, 0:1],
                        pst_tile,
                        ones_col[:, 0:1],
                        start=(kt == 0),
                        stop=(kt == NS - 1),
                    )
                    for c0, cw in _chunks(D, NCH):
                        nc.tensor.matmul(
                            ops[:, c0 : c0 + cw],
                            pst_tile,
                            V[:, kt, c0 : c0 + cw],
                            start=(kt == 0),
                            stop=(kt == NS - 1),
                        )
                rden = den_pool.tile([P, 1], f32, tag="rden")
                nc.vector.reciprocal(rden[:, 0:1], dps[:, 0:1])
                ot = o_pool.tile([P, D], f32, tag="o")
                nc.scalar.activation(ot[:, :], ops[:, 0:D], AF.Copy, scale=rden[:, 0:1])
                nc.sync.dma_start(out=out_d[qt * P : (qt + 1) * P, :], in_=ot[:, :])

    nc.compile()
    return nc


_NC_CACHE = {}


def _get_nc(key=(2048, 1024, 1024, 1024)):
    if key not in _NC_CACHE:
        _NC_CACHE[key] = build_nc(*key)
    return _NC_CACHE[key]


def shard_inputs(x, mask, ws):
    """Build per-core input maps. Odd cores get the key axis rotated by Sq so
    their query half sits at rows [0:Sq] (softmax/PV are key-order invariant)."""
    Sq = x.shape[1] // 2
    in_maps = []
    for c in range(N_CORES):
        b, h = c // 2, c % 2
        if h == 0:
            xc = x[b]
            mc = mask[b, :Sq, :]
        else:
            xc = np.concatenate([x[b, Sq:], x[b, :Sq]], axis=0)
            mc = np.concatenate([mask[b, Sq:, Sq:], mask[b, Sq:, :Sq]], axis=1)
        in_maps.append(
            {
                "x": np.ascontiguousarray(xc),
                "mask": np.ascontiguousarray(mc),
                **ws,
            }
        )
    return in_maps


def kernel(**inputs):
    """Full-problem entry point: full unsharded inputs -> full output."""
    from concourse.bass_utils import run_bass_kernel_spmd

    x = np.asarray(inputs["x"], dtype=np.float32)
    mask = np.asarray(inputs["mask"], dtype=np.int32)
    ws = {
        k: np.ascontiguousarray(np.asarray(inputs[k], dtype=np.float32))
        for k in ("Wq", "bq", "Wk", "bk", "Wv", "bv")
    }

    nc = _get_nc()
    in_maps = shard_inputs(x, mask, ws)
    res = run_bass_kernel_spmd(nc, in_maps, core_ids=list(range(N_CORES)))

    Sq = S_FULL // 2
    out = np.empty((B, S_FULL, QD), dtype=np.float32)
    for c, r in enumerate(res.results):
        b, h = c // 2, c % 2
        out[b, h * Sq : (h + 1) * Sq, :] = r["out"]
    return out
